# revision 9
# baseline (speedup 1.0000x reference)
"""nn_MergeWindows — Trainium2 Bass kernel (8 NeuronCores, SPMD over image rows).

The reference's output is out[b,c,y,x] = 1.0 iff remap[argmax_d masks[b,d,y,x]]
== c, where remap merges channels according to a scan over tiny metadata
(boundary-strip argmaxes + slot-feature cosine sims).  remap is computed on
the host in microseconds; the heavy per-pixel work (argmax over 32 channels +
relabel + one-hot; 128 MiB in / 128 MiB out) runs on 8 NeuronCores, each
handling 128 of the 1024 rows.  This puts the kernel at the HBM roofline:
~4.2 MiB in + 4.2 MiB out per core per tile.

Device pipeline per [128 rows, 32 ch, 256 cols] tile — DVE only, plus a few
tiny ACT memsets (the previous version's 256 GpSimd is_equal insts at ~2.2us
each were the bottleneck, 560us of a 654us span):
  1. mx  = pairwise max tree over channels (5 contiguous tensor_tensor max
     stages; a strided tensor_reduce measures 1.65 cyc/elem vs 1.0 here)
  2. oh  = is_equal(masks, mx broadcast)  -> one-hot [128, 32, 256] in one TT
  3. per merged channel pair (compile-time-specialized from remap):
     oh[:, keep, :] += oh[:, rem, :]   (DVE, 256-elem TT)
     oh[:, rem, :]   = 0               (ACT, scale=0 activation)
  4. DMA out.

is_equal single-fires only if no two channels tie at the per-pixel max in
f32.  The host pass bumps the first tied winner by 1 ulp at the (rare) tie
pixels before upload, which preserves argmax-with-first-match semantics
exactly, so the device compare is exact.

The program is compiled once per distinct remap pattern (the merge fixups are
baked in) and cached.
"""

import json

import numpy as np

N_WINDOWS = 4
WIN_H = WIN_W = 512
IMG_H = IMG_W = 1024
C = 32
MPW = C // N_WINDOWS
SLOT_DIM = 64
SIM_THRESH = 0.1

N_CORES = 8
ROWS_PER_CORE = IMG_H // N_CORES  # 128
G = 256          # column-tile width (1 KiB DMA descriptor lines)
NTILES = IMG_W // G

_cache = {}


# --------------------------------------------------------------------------
# host-side merge decision (mirrors reference._merge_windows metadata math)
# --------------------------------------------------------------------------
def _compute_remap(masks, slot_features, pl, pt):
    B, Ch, H, W = masks.shape
    mpw = Ch // N_WINDOWS
    ranges = [(i * mpw, (i + 1) * mpw) for i in range(N_WINDOWS)]

    adjacency = []
    for i in range(N_WINDOWS):
        for j in range(i + 1, N_WINDOWS):
            if pt[i] == pt[j] and abs(pl[i] - pl[j]) == WIN_W:
                adjacency.append((i, j, True) if pl[i] < pl[j] else (j, i, True))
            if pl[i] == pl[j] and abs(pt[i] - pt[j]) == WIN_H:
                adjacency.append((i, j, False) if pt[i] < pt[j] else (j, i, False))

    edge_l = np.zeros(Ch, bool)
    edge_r = np.zeros(Ch, bool)
    edge_t = np.zeros(Ch, bool)
    edge_b = np.zeros(Ch, bool)
    m0 = masks[0]
    for wi, (s, e) in enumerate(ranges):
        ys, ye = max(pt[wi], 0), min(pt[wi] + WIN_H, H)
        xs, xe = max(pl[wi], 0), min(pl[wi] + WIN_W, W)
        if ys >= ye or xs >= xe:
            continue
        ids_l = np.argmax(m0[:, ys:ye, xs], axis=0)
        ids_r = np.argmax(m0[:, ys:ye, xe - 1], axis=0)
        ids_t = np.argmax(m0[:, ys, xs:xe], axis=0)
        ids_b = np.argmax(m0[:, ye - 1, xs:xe], axis=0)
        for k in range(s, e):
            edge_l[k] = np.any(ids_l == k)
            edge_r[k] = np.any(ids_r == k)
            edge_t[k] = np.any(ids_t == k)
            edge_b[k] = np.any(ids_b == k)

    ci_l, cj_l, wi_l, wj_l, hz_l = [], [], [], [], []
    for wi, wj, horiz in adjacency:
        si, ei = ranges[wi]
        sj, ej = ranges[wj]
        for ci in range(si + 1, ei):
            for cj in range(sj + 1, ej):
                ci_l.append(ci)
                cj_l.append(cj)
                wi_l.append(wi)
                wj_l.append(wj)
                hz_l.append(horiz)

    target = np.arange(Ch)
    if not ci_l:
        return target

    sf = np.asarray(slot_features, np.float32)
    sf_n = sf / (np.linalg.norm(sf, axis=-1, keepdims=True) + np.float32(1e-8))
    ci_a = np.array(ci_l)
    cj_a = np.array(cj_l)
    rel_i = ci_a % mpw - 1
    rel_j = cj_a % mpw - 1
    fi = sf_n[np.array(wi_l), rel_i]
    fj = sf_n[np.array(wj_l), rel_j]
    sims = np.sum(fi * fj, axis=-1)
    hz = np.array(hz_l)
    edge_ok = np.where(hz, edge_r[ci_a] & edge_l[cj_a], edge_b[ci_a] & edge_t[cj_a])
    passing = edge_ok & (sims > np.float32(SIM_THRESH))

    merged = np.zeros(Ch, bool)
    for ci, cj, ok in zip(ci_l, cj_l, passing):
        if ok and not merged[ci] and not merged[cj]:
            keep, rem = min(ci, cj), max(ci, cj)
            target[target == rem] = keep
            merged[rem] = True
    return target


# --------------------------------------------------------------------------
# wait-split post-pass: the pinned neuronxcc allows only ONE sync wait per
# instruction; hoist extras onto preceding same-engine EventSemaphore insts.
# --------------------------------------------------------------------------
def _split_excess_waits(bir_json_bytes, limit=1):
    j = json.loads(bir_json_bytes)
    counter = [0]
    for fn in j.get("functions", []):
        for bb in fn.get("blocks", []):
            new_insts = []
            for inst in bb.get("instructions", []):
                si = inst.get("sync_info") or {}
                waits = si.get("on_wait") or []
                if len(waits) > limit:
                    extra = waits[: len(waits) - limit]
                    si["on_wait"] = waits[len(waits) - limit:]
                    inst["sync_info"] = si
                    for i in range(0, len(extra), limit):
                        counter[0] += 1
                        new_insts.append({
                            "engine": inst["engine"],
                            "ins": [],
                            "name": f"{inst['name']}_hoistw{counter[0]}",
                            "opcode": "EventSemaphore",
                            "outs": [],
                            "sync_info": {"on_update": [],
                                          "on_wait": extra[i: i + limit]},
                        })
                new_insts.append(inst)
            bb["instructions"] = new_insts
    return json.dumps(j).encode()


def _build_program(remap_key):
    if remap_key in _cache:
        return _cache[remap_key]

    import concourse.bass as bass
    import concourse.tile as tile
    from concourse import mybir

    remap = list(remap_key)
    # out[c] = sum_{d: remap[d]==c} oh0[d]; channels with remap[d] != d are
    # zeroed.  remap is chain-free (fixed point on keeps).
    adds = [(int(remap[d]), d) for d in range(C) if remap[d] != d]
    rems = [d for d in range(C) if remap[d] != d]

    # batch adds: same delta (rem-keep) + uniform rem stride -> one 3D-AP TT
    def _batch_adds(pairs):
        from collections import defaultdict
        bydelta = defaultdict(list)
        for keep, rem in pairs:
            bydelta[rem - keep].append((keep, rem))
        groups = []
        for delta in sorted(bydelta):
            run = sorted(bydelta[delta], key=lambda p: p[1])
            i = 0
            while i < len(run):
                j = i + 1
                stride = None
                while j < len(run):
                    s = run[j][1] - run[j - 1][1]
                    if stride is None:
                        stride = s
                    if s != stride:
                        break
                    j += 1
                groups.append((run[i:j], stride if j - i > 1 else 1))
                i = j
        return groups

    add_groups = _batch_adds(adds)

    # batch zeros: maximal uniform-stride runs over sorted rems
    def _batch_runs(chans):
        chans = sorted(chans)
        groups = []
        i = 0
        while i < len(chans):
            j = i + 1
            stride = None
            while j < len(chans):
                s = chans[j] - chans[j - 1]
                if stride is None:
                    stride = s
                if s != stride:
                    break
                j += 1
            groups.append((chans[i:j], stride if j - i > 1 else 1))
            i = j
        return groups

    zero_groups = _batch_runs(rems)

    f32 = mybir.dt.float32
    nc = bass.Bass()
    masks_in = nc.dram_tensor("masks", [C, ROWS_PER_CORE, IMG_W], f32,
                              kind="ExternalInput")
    out_dram = nc.dram_tensor("out", [C, ROWS_PER_CORE, IMG_W], f32,
                              kind="ExternalOutput")

    def _chan_slice_ap(tile_ap, chans, stride):
        # AP over out_tile channels {chans[0], chans[0]+stride, ...} x [G]
        base = tile_ap[:, chans[0], :]
        ch_stride = base.ap[-1][0] * G * stride
        return bass.AP(tensor=base.tensor, offset=base.offset,
                       ap=[base.ap[0], [ch_stride, len(chans)], base.ap[-1]])

    with tile.TileContext(nc) as tc:
        with (
            tc.tile_pool(name="inp", bufs=3) as inp,
            tc.tile_pool(name="outp", bufs=2) as outp,
            tc.tile_pool(name="work", bufs=1) as work,
        ):
            for t in range(NTILES):
                sl = slice(G * t, G * (t + 1))
                in_tile = inp.tile([128, C, G], f32, tag="in_tile")
                t8a = work.tile([128, 8, G], f32, tag="t8a")
                t8b = work.tile([128, 8, G], f32, tag="t8b")
                m8 = work.tile([128, 8, G], f32, tag="m8")
                m4 = work.tile([128, 4, G], f32, tag="m4")
                m2 = work.tile([128, 2, G], f32, tag="m2")
                mx = work.tile([128, G], f32, tag="mx")
                TT = nc.vector.tensor_tensor
                MAX = mybir.AluOpType.max
                if t == 0:
                    # first tile: quarter loads (parallel descriptor gen)
                    for q in range(4):
                        cq = slice(8 * q, 8 * (q + 1))
                        nc.sync.dma_start(
                            in_tile[:, cq, :],
                            masks_in[cq, :, sl].rearrange("d p g -> p d g"))
                else:
                    nc.sync.dma_start(
                        in_tile[:],
                        masks_in[:, :, sl].rearrange("d p g -> p d g"))

                # pairwise max tree (contiguous innermost)
                TT(out=t8a[:], in0=in_tile[:, 0:8, :],
                   in1=in_tile[:, 8:16, :], op=MAX)
                TT(out=t8b[:], in0=in_tile[:, 16:24, :],
                   in1=in_tile[:, 24:32, :], op=MAX)
                TT(out=m8[:], in0=t8a[:], in1=t8b[:], op=MAX)
                TT(out=m4[:], in0=m8[:, 0:4, :], in1=m8[:, 4:8, :], op=MAX)
                TT(out=m2[:], in0=m4[:, 0:2, :], in1=m4[:, 2:4, :], op=MAX)
                TT(out=mx[:], in0=m2[:, 0, :], in1=m2[:, 1, :], op=MAX)

                # one-hot: is_equal against broadcast max (exact f32 compare;
                # host pre-pass guarantees a unique per-pixel winner)
                out_tile = outp.tile([128, C, G], f32, tag="out_tile")
                mx_ap = mx[:]
                mx_b = bass.AP(tensor=mx_ap.tensor, offset=mx_ap.offset,
                               ap=[mx_ap.ap[0], [0, C], mx_ap.ap[-1]])
                nc.vector.tensor_tensor(out=out_tile[:], in0=in_tile[:],
                                        in1=mx_b,
                                        op=mybir.AluOpType.is_equal)

                # channel merges (baked in from remap), batched by stride
                for pairs, stride in add_groups:
                    keeps = [p[0] for p in pairs]
                    rms = [p[1] for p in pairs]
                    kap = _chan_slice_ap(out_tile, keeps, stride)
                    rap = _chan_slice_ap(out_tile, rms, stride)
                    nc.vector.tensor_tensor(out=kap, in0=kap, in1=rap,
                                            op=mybir.AluOpType.add)
                for chans, stride in zero_groups:
                    zap = _chan_slice_ap(out_tile, chans, stride)
                    nc.scalar.activation(
                        zap, zap,
                        mybir.ActivationFunctionType.Identity, scale=0.0)

                if t == NTILES - 1:
                    # last tile: channel-quartered stores so the tail is
                    # gen+transfer of 1 MiB pieces instead of one 4 MiB blob
                    for q in range(4):
                        cq = slice(8 * q, 8 * (q + 1))
                        nc.sync.dma_start(
                            out_dram[cq, :, sl].rearrange("c p g -> p c g"),
                            out_tile[:, cq, :])
                else:
                    nc.sync.dma_start(
                        out_dram[:, :, sl].rearrange("c p g -> p c g"),
                        out_tile[:])

    orig = nc.to_json_bytes
    nc.to_json_bytes = lambda: _split_excess_waits(orig())
    _cache[remap_key] = nc
    return nc


def kernel(masks, slot_features, pad_left, pad_top):
    from concourse.bass_utils import run_bass_kernel_spmd

    masks = np.asarray(masks, np.float32)
    slot_features = np.asarray(slot_features, np.float32)
    pl = [int(v) for v in np.asarray(pad_left)]
    pt = [int(v) for v in np.asarray(pad_top)]

    remap = _compute_remap(masks, slot_features, pl, pt)

    # tie pre-fix: where >1 channel equals the per-pixel max, bump the first
    # (= reference argmax winner) by 1 ulp so the device is_equal single-fires
    m0 = masks[0]
    mxh = m0.max(axis=0)
    eq = m0 == mxh[None]
    nties = int((eq.sum(axis=0) > 1).sum())
    if nties:
        masks = masks.copy()
        m0 = masks[0]
        ys, xs = np.nonzero(eq.sum(axis=0) > 1)
        for y, x in zip(ys, xs):
            d0 = int(np.argmax(eq[:, y, x]))
            v = m0[d0, y, x]
            m0[d0, y, x] = np.nextafter(v, np.float32(np.inf), dtype=np.float32)

    nc = _build_program(tuple(int(v) for v in remap))
    in_maps = []
    for i in range(N_CORES):
        slab = np.ascontiguousarray(
            masks[0, :, i * ROWS_PER_CORE:(i + 1) * ROWS_PER_CORE, :])
        in_maps.append({"masks": slab})

    res = run_bass_kernel_spmd(nc, in_maps, core_ids=list(range(N_CORES)))

    out = np.empty((1, C, IMG_H, IMG_W), np.float32)
    for i, r in enumerate(res.results):
        out[0, :, i * ROWS_PER_CORE:(i + 1) * ROWS_PER_CORE, :] = r["out"]
    return out


# revision 12
# speedup vs baseline: 1.0152x; 1.0152x over previous
"""nn_MergeWindows — Trainium2 Bass kernel (8 NeuronCores, SPMD over image rows).

The reference's output is out[b,c,y,x] = 1.0 iff remap[argmax_d masks[b,d,y,x]]
== c, where remap merges channels according to a scan over tiny metadata
(boundary-strip argmaxes + slot-feature cosine sims).  remap is computed on
the host in microseconds; the heavy per-pixel work (argmax over 32 channels +
relabel + one-hot; 128 MiB in / 128 MiB out) runs on 8 NeuronCores, each
handling 128 of the 1024 rows.  This puts the kernel at the HBM roofline:
~4.2 MiB in + 4.2 MiB out per core per tile.

Device pipeline per [128 rows, 32 ch, 256 cols] tile — DVE only, plus a few
tiny ACT memsets (the previous version's 256 GpSimd is_equal insts at ~2.2us
each were the bottleneck, 560us of a 654us span):
  1. mx  = pairwise max tree over channels (5 contiguous tensor_tensor max
     stages; a strided tensor_reduce measures 1.65 cyc/elem vs 1.0 here)
  2. oh  = is_equal(masks, mx broadcast)  -> one-hot [128, 32, 256] in one TT
  3. per merged channel pair (compile-time-specialized from remap):
     oh[:, keep, :] += oh[:, rem, :]   (DVE, 256-elem TT)
     oh[:, rem, :]   = 0               (ACT, scale=0 activation)
  4. DMA out.

is_equal single-fires only if no two channels tie at the per-pixel max in
f32.  The host pass bumps the first tied winner by 1 ulp at the (rare) tie
pixels before upload, which preserves argmax-with-first-match semantics
exactly, so the device compare is exact.

The program is compiled once per distinct remap pattern (the merge fixups are
baked in) and cached.
"""

import json

import numpy as np

N_WINDOWS = 4
WIN_H = WIN_W = 512
IMG_H = IMG_W = 1024
C = 32
MPW = C // N_WINDOWS
SLOT_DIM = 64
SIM_THRESH = 0.1

N_CORES = 8
ROWS_PER_CORE = IMG_H // N_CORES  # 128
G = 256          # column-tile width (1 KiB DMA descriptor lines)
NTILES = IMG_W // G

_cache = {}


# --------------------------------------------------------------------------
# host-side merge decision (mirrors reference._merge_windows metadata math)
# --------------------------------------------------------------------------
def _compute_remap(masks, slot_features, pl, pt):
    B, Ch, H, W = masks.shape
    mpw = Ch // N_WINDOWS
    ranges = [(i * mpw, (i + 1) * mpw) for i in range(N_WINDOWS)]

    adjacency = []
    for i in range(N_WINDOWS):
        for j in range(i + 1, N_WINDOWS):
            if pt[i] == pt[j] and abs(pl[i] - pl[j]) == WIN_W:
                adjacency.append((i, j, True) if pl[i] < pl[j] else (j, i, True))
            if pl[i] == pl[j] and abs(pt[i] - pt[j]) == WIN_H:
                adjacency.append((i, j, False) if pt[i] < pt[j] else (j, i, False))

    edge_l = np.zeros(Ch, bool)
    edge_r = np.zeros(Ch, bool)
    edge_t = np.zeros(Ch, bool)
    edge_b = np.zeros(Ch, bool)
    m0 = masks[0]
    for wi, (s, e) in enumerate(ranges):
        ys, ye = max(pt[wi], 0), min(pt[wi] + WIN_H, H)
        xs, xe = max(pl[wi], 0), min(pl[wi] + WIN_W, W)
        if ys >= ye or xs >= xe:
            continue
        ids_l = np.argmax(m0[:, ys:ye, xs], axis=0)
        ids_r = np.argmax(m0[:, ys:ye, xe - 1], axis=0)
        ids_t = np.argmax(m0[:, ys, xs:xe], axis=0)
        ids_b = np.argmax(m0[:, ye - 1, xs:xe], axis=0)
        for k in range(s, e):
            edge_l[k] = np.any(ids_l == k)
            edge_r[k] = np.any(ids_r == k)
            edge_t[k] = np.any(ids_t == k)
            edge_b[k] = np.any(ids_b == k)

    ci_l, cj_l, wi_l, wj_l, hz_l = [], [], [], [], []
    for wi, wj, horiz in adjacency:
        si, ei = ranges[wi]
        sj, ej = ranges[wj]
        for ci in range(si + 1, ei):
            for cj in range(sj + 1, ej):
                ci_l.append(ci)
                cj_l.append(cj)
                wi_l.append(wi)
                wj_l.append(wj)
                hz_l.append(horiz)

    target = np.arange(Ch)
    if not ci_l:
        return target

    sf = np.asarray(slot_features, np.float32)
    sf_n = sf / (np.linalg.norm(sf, axis=-1, keepdims=True) + np.float32(1e-8))
    ci_a = np.array(ci_l)
    cj_a = np.array(cj_l)
    rel_i = ci_a % mpw - 1
    rel_j = cj_a % mpw - 1
    fi = sf_n[np.array(wi_l), rel_i]
    fj = sf_n[np.array(wj_l), rel_j]
    sims = np.sum(fi * fj, axis=-1)
    hz = np.array(hz_l)
    edge_ok = np.where(hz, edge_r[ci_a] & edge_l[cj_a], edge_b[ci_a] & edge_t[cj_a])
    passing = edge_ok & (sims > np.float32(SIM_THRESH))

    merged = np.zeros(Ch, bool)
    for ci, cj, ok in zip(ci_l, cj_l, passing):
        if ok and not merged[ci] and not merged[cj]:
            keep, rem = min(ci, cj), max(ci, cj)
            target[target == rem] = keep
            merged[rem] = True
    return target


# --------------------------------------------------------------------------
# wait-split post-pass: the pinned neuronxcc allows only ONE sync wait per
# instruction; hoist extras onto preceding same-engine EventSemaphore insts.
# --------------------------------------------------------------------------
def _split_excess_waits(bir_json_bytes, limit=1):
    j = json.loads(bir_json_bytes)
    counter = [0]
    for fn in j.get("functions", []):
        for bb in fn.get("blocks", []):
            new_insts = []
            for inst in bb.get("instructions", []):
                si = inst.get("sync_info") or {}
                waits = si.get("on_wait") or []
                if len(waits) > limit:
                    extra = waits[: len(waits) - limit]
                    si["on_wait"] = waits[len(waits) - limit:]
                    inst["sync_info"] = si
                    for i in range(0, len(extra), limit):
                        counter[0] += 1
                        new_insts.append({
                            "engine": inst["engine"],
                            "ins": [],
                            "name": f"{inst['name']}_hoistw{counter[0]}",
                            "opcode": "EventSemaphore",
                            "outs": [],
                            "sync_info": {"on_update": [],
                                          "on_wait": extra[i: i + limit]},
                        })
                new_insts.append(inst)
            bb["instructions"] = new_insts
    return json.dumps(j).encode()


def _build_program(remap_key):
    if remap_key in _cache:
        return _cache[remap_key]

    import concourse.bass as bass
    import concourse.tile as tile
    from concourse import mybir

    remap = list(remap_key)
    # out[c] = sum_{d: remap[d]==c} oh0[d]; channels with remap[d] != d are
    # zeroed.  remap is chain-free (fixed point on keeps).
    adds = [(int(remap[d]), d) for d in range(C) if remap[d] != d]
    rems = [d for d in range(C) if remap[d] != d]

    # batch adds: any subset whose keeps AND rems each form an arithmetic
    # progression (nonzero keep stride) can be one multi-channel TT.
    # Greedy: longest APs first, then pair up leftovers.
    def _batch_adds(pairs):
        remaining = sorted(pairs, key=lambda p: p[1])
        groups = []
        # try AP runs of length >= 2 (greedy longest-first)
        while remaining:
            best = None
            n = len(remaining)
            for i in range(n):
                for j in range(i + 1, n):
                    k0, r0 = remaining[i]
                    k1, r1 = remaining[j]
                    sk, sr = k1 - k0, r1 - r0
                    if sk == 0:
                        continue
                    run = [remaining[i], remaining[j]]
                    ck, cr = k1, r1
                    for l in range(j + 1, n):
                        k2, r2 = remaining[l]
                        if k2 == ck + sk and r2 == cr + sr:
                            run.append(remaining[l])
                            ck, cr = k2, r2
                    if best is None or len(run) > len(best[0]):
                        best = (run, sk, sr)
            if best is None or len(best[0]) < 2:
                break
            run, sk, sr = best
            groups.append((run, sk, sr))
            for p in run:
                remaining.remove(p)
        for p in remaining:
            groups.append(([p], 1, 1))
        return groups

    add_groups = _batch_adds(adds)

    # batch zeros: maximal uniform-stride runs over sorted rems
    def _batch_runs(chans):
        chans = sorted(chans)
        groups = []
        i = 0
        while i < len(chans):
            j = i + 1
            stride = None
            while j < len(chans):
                s = chans[j] - chans[j - 1]
                if stride is None:
                    stride = s
                if s != stride:
                    break
                j += 1
            groups.append((chans[i:j], stride if j - i > 1 else 1))
            i = j
        return groups

    zero_groups = _batch_runs(rems)

    f32 = mybir.dt.float32
    nc = bass.Bass()
    masks_in = nc.dram_tensor("masks", [C, ROWS_PER_CORE, IMG_W], f32,
                              kind="ExternalInput")
    out_dram = nc.dram_tensor("out", [C, ROWS_PER_CORE, IMG_W], f32,
                              kind="ExternalOutput")

    def _chan_slice_ap(tile_ap, chans, stride):
        # AP over out_tile channels {chans[0], chans[0]+stride, ...} x [G]
        base = tile_ap[:, chans[0], :]
        ch_stride = base.ap[-1][0] * G * stride
        return bass.AP(tensor=base.tensor, offset=base.offset,
                       ap=[base.ap[0], [ch_stride, len(chans)], base.ap[-1]])

    with tile.TileContext(nc) as tc:
        with (
            tc.tile_pool(name="inp", bufs=3) as inp,
            tc.tile_pool(name="outp", bufs=2) as outp,
            tc.tile_pool(name="work", bufs=1) as work,
        ):
            for t in range(NTILES):
                sl = slice(G * t, G * (t + 1))
                in_tile = inp.tile([128, C, G], f32, tag="in_tile")
                t8a = work.tile([128, 8, G], f32, tag="t8a")
                t8b = work.tile([128, 8, G], f32, tag="t8b")
                m8 = work.tile([128, 8, G], f32, tag="m8")
                m4 = work.tile([128, 4, G], f32, tag="m4")
                m2 = work.tile([128, 2, G], f32, tag="m2")
                mx = work.tile([128, G], f32, tag="mx")
                TT = nc.vector.tensor_tensor
                MAX = mybir.AluOpType.max
                if t == 0:
                    # first tile: quarter loads + quarters-first tree; each
                    # quarter's compute waits only its own DMA, so the DVE
                    # starts as soon as the first 1 MiB lands
                    for q in range(4):
                        cq = slice(8 * q, 8 * (q + 1))
                        nc.sync.dma_start(
                            in_tile[:, cq, :],
                            masks_in[cq, :, sl].rearrange("d p g -> p d g"))
                    q2v = [t8a[:, 0:4, :], t8a[:, 4:8, :],
                           t8b[:, 0:4, :], t8b[:, 4:8, :]]
                    q1v = [m8[:, 0:2, :], m8[:, 2:4, :],
                           m8[:, 4:6, :], m8[:, 6:8, :]]
                    for q in range(4):
                        TT(out=q2v[q], in0=in_tile[:, 8 * q:8 * q + 4, :],
                           in1=in_tile[:, 8 * q + 4:8 * q + 8, :], op=MAX)
                        TT(out=q1v[q], in0=q2v[q][:, 0:2, :],
                           in1=q2v[q][:, 2:4, :], op=MAX)
                        TT(out=m4[:, q, :], in0=q1v[q][:, 0, :],
                           in1=q1v[q][:, 1, :], op=MAX)
                    TT(out=m2[:], in0=m4[:, 0:2, :], in1=m4[:, 2:4, :], op=MAX)
                    TT(out=mx[:], in0=m2[:, 0, :], in1=m2[:, 1, :], op=MAX)
                else:
                    nc.sync.dma_start(
                        in_tile[:],
                        masks_in[:, :, sl].rearrange("d p g -> p d g"))

                    # pairwise max tree (contiguous innermost)
                    TT(out=t8a[:], in0=in_tile[:, 0:8, :],
                       in1=in_tile[:, 8:16, :], op=MAX)
                    TT(out=t8b[:], in0=in_tile[:, 16:24, :],
                       in1=in_tile[:, 24:32, :], op=MAX)
                    TT(out=m8[:], in0=t8a[:], in1=t8b[:], op=MAX)
                    TT(out=m4[:], in0=m8[:, 0:4, :], in1=m8[:, 4:8, :], op=MAX)
                    TT(out=m2[:], in0=m4[:, 0:2, :], in1=m4[:, 2:4, :], op=MAX)
                    TT(out=mx[:], in0=m2[:, 0, :], in1=m2[:, 1, :], op=MAX)

                # one-hot: is_equal against broadcast max (exact f32 compare;
                # host pre-pass guarantees a unique per-pixel winner)
                out_tile = outp.tile([128, C, G], f32, tag="out_tile")
                mx_ap = mx[:]
                mx_b = bass.AP(tensor=mx_ap.tensor, offset=mx_ap.offset,
                               ap=[mx_ap.ap[0], [0, C], mx_ap.ap[-1]])
                nc.vector.tensor_tensor(out=out_tile[:], in0=in_tile[:],
                                        in1=mx_b,
                                        op=mybir.AluOpType.is_equal)

                # channel merges (baked in from remap), batched by stride
                for pairs, sk, sr in add_groups:
                    keeps = [p[0] for p in pairs]
                    rms = [p[1] for p in pairs]
                    kap = _chan_slice_ap(out_tile, keeps, sk)
                    rap = _chan_slice_ap(out_tile, rms, sr)
                    nc.vector.tensor_tensor(out=kap, in0=kap, in1=rap,
                                            op=mybir.AluOpType.add)
                for chans, stride in zero_groups:
                    zap = _chan_slice_ap(out_tile, chans, stride)
                    nc.scalar.activation(
                        zap, zap,
                        mybir.ActivationFunctionType.Identity, scale=0.0)

                if t == NTILES - 1:
                    # last tile: channel-quartered stores so the tail is
                    # gen+transfer of 1 MiB pieces instead of one 4 MiB blob
                    for q in range(4):
                        cq = slice(8 * q, 8 * (q + 1))
                        nc.sync.dma_start(
                            out_dram[cq, :, sl].rearrange("c p g -> p c g"),
                            out_tile[:, cq, :])
                else:
                    nc.sync.dma_start(
                        out_dram[:, :, sl].rearrange("c p g -> p c g"),
                        out_tile[:])

    orig = nc.to_json_bytes
    nc.to_json_bytes = lambda: _split_excess_waits(orig())
    _cache[remap_key] = nc
    return nc


def kernel(masks, slot_features, pad_left, pad_top):
    from concourse.bass_utils import run_bass_kernel_spmd

    masks = np.asarray(masks, np.float32)
    slot_features = np.asarray(slot_features, np.float32)
    pl = [int(v) for v in np.asarray(pad_left)]
    pt = [int(v) for v in np.asarray(pad_top)]

    remap = _compute_remap(masks, slot_features, pl, pt)

    # tie pre-fix: where >1 channel equals the per-pixel max, bump the first
    # (= reference argmax winner) by 1 ulp so the device is_equal single-fires
    m0 = masks[0]
    mxh = m0.max(axis=0)
    eq = m0 == mxh[None]
    nties = int((eq.sum(axis=0) > 1).sum())
    if nties:
        masks = masks.copy()
        m0 = masks[0]
        ys, xs = np.nonzero(eq.sum(axis=0) > 1)
        for y, x in zip(ys, xs):
            d0 = int(np.argmax(eq[:, y, x]))
            v = m0[d0, y, x]
            m0[d0, y, x] = np.nextafter(v, np.float32(np.inf), dtype=np.float32)

    nc = _build_program(tuple(int(v) for v in remap))
    in_maps = []
    for i in range(N_CORES):
        slab = np.ascontiguousarray(
            masks[0, :, i * ROWS_PER_CORE:(i + 1) * ROWS_PER_CORE, :])
        in_maps.append({"masks": slab})

    res = run_bass_kernel_spmd(nc, in_maps, core_ids=list(range(N_CORES)))

    out = np.empty((1, C, IMG_H, IMG_W), np.float32)
    for i, r in enumerate(res.results):
        out[0, :, i * ROWS_PER_CORE:(i + 1) * ROWS_PER_CORE, :] = r["out"]
    return out


# revision 13
# speedup vs baseline: 1.0440x; 1.0284x over previous
"""nn_MergeWindows — Trainium2 Bass kernel (8 NeuronCores, SPMD over image rows).

The reference's output is out[b,c,y,x] = 1.0 iff remap[argmax_d masks[b,d,y,x]]
== c, where remap merges channels according to a scan over tiny metadata
(boundary-strip argmaxes + slot-feature cosine sims).  remap is computed on
the host in microseconds; the heavy per-pixel work (argmax over 32 channels +
relabel + one-hot; 128 MiB in / 128 MiB out) runs on 8 NeuronCores, each
handling 128 of the 1024 rows.  This puts the kernel at the HBM roofline:
~4.2 MiB in + 4.2 MiB out per core per tile.

Device pipeline per [128 rows, 32 ch, 256 cols] tile — DVE only, plus a few
tiny ACT memsets (the original version's 256 GpSimd is_equal insts at ~2.2us
each were the bottleneck, 560us of a 654us span):
  1. mx  = pairwise max tree over channels (contiguous tensor_tensor max
     stages; a strided tensor_reduce measures 1.65 cyc/elem vs 1.0 here).
     Tile 0 is loaded as four 8-channel quarter-DMAs with a quarters-first
     tree so the DVE starts as soon as the first 1 MiB lands (~13us ramp,
     descriptor-gen + HBM-bandwidth bound).
  2. oh  = is_equal(masks, mx broadcast)  -> one-hot [128, 32, 256] in one TT
  3. merge fixups (compile-time-specialized from remap, batched into
     multi-channel strided-AP instructions):
     oh[:, keep, :] += oh[:, rem, :]   (DVE TT, keeps/rems each an AP run)
     oh[:, rem, :]   = 0               (ACT, scale=0 activation, batched)
  4. DMA out (the last tile stores in channel quarters to trim the tail).

Measured: 654,528 ns (baseline) -> 124,825 ns on 8 axon trn2 cores; DVE busy
~93us (saturated), DMA active ~107us, HBM roofline for 32 MiB/core ~94us.

is_equal single-fires only if no two channels tie at the per-pixel max in
f32.  The host pass bumps the first tied winner by 1 ulp at the (rare) tie
pixels before upload, which preserves argmax-with-first-match semantics
exactly, so the device compare is exact.

The program is compiled once per distinct remap pattern (the merge fixups are
baked in) and cached.
"""

import json

import numpy as np

N_WINDOWS = 4
WIN_H = WIN_W = 512
IMG_H = IMG_W = 1024
C = 32
MPW = C // N_WINDOWS
SLOT_DIM = 64
SIM_THRESH = 0.1

N_CORES = 8
ROWS_PER_CORE = IMG_H // N_CORES  # 128
G = 256          # column-tile width (1 KiB DMA descriptor lines)
NTILES = IMG_W // G

_cache = {}


# --------------------------------------------------------------------------
# host-side merge decision (mirrors reference._merge_windows metadata math)
# --------------------------------------------------------------------------
def _compute_remap(masks, slot_features, pl, pt):
    B, Ch, H, W = masks.shape
    mpw = Ch // N_WINDOWS
    ranges = [(i * mpw, (i + 1) * mpw) for i in range(N_WINDOWS)]

    adjacency = []
    for i in range(N_WINDOWS):
        for j in range(i + 1, N_WINDOWS):
            if pt[i] == pt[j] and abs(pl[i] - pl[j]) == WIN_W:
                adjacency.append((i, j, True) if pl[i] < pl[j] else (j, i, True))
            if pl[i] == pl[j] and abs(pt[i] - pt[j]) == WIN_H:
                adjacency.append((i, j, False) if pt[i] < pt[j] else (j, i, False))

    edge_l = np.zeros(Ch, bool)
    edge_r = np.zeros(Ch, bool)
    edge_t = np.zeros(Ch, bool)
    edge_b = np.zeros(Ch, bool)
    m0 = masks[0]
    for wi, (s, e) in enumerate(ranges):
        ys, ye = max(pt[wi], 0), min(pt[wi] + WIN_H, H)
        xs, xe = max(pl[wi], 0), min(pl[wi] + WIN_W, W)
        if ys >= ye or xs >= xe:
            continue
        ids_l = np.argmax(m0[:, ys:ye, xs], axis=0)
        ids_r = np.argmax(m0[:, ys:ye, xe - 1], axis=0)
        ids_t = np.argmax(m0[:, ys, xs:xe], axis=0)
        ids_b = np.argmax(m0[:, ye - 1, xs:xe], axis=0)
        for k in range(s, e):
            edge_l[k] = np.any(ids_l == k)
            edge_r[k] = np.any(ids_r == k)
            edge_t[k] = np.any(ids_t == k)
            edge_b[k] = np.any(ids_b == k)

    ci_l, cj_l, wi_l, wj_l, hz_l = [], [], [], [], []
    for wi, wj, horiz in adjacency:
        si, ei = ranges[wi]
        sj, ej = ranges[wj]
        for ci in range(si + 1, ei):
            for cj in range(sj + 1, ej):
                ci_l.append(ci)
                cj_l.append(cj)
                wi_l.append(wi)
                wj_l.append(wj)
                hz_l.append(horiz)

    target = np.arange(Ch)
    if not ci_l:
        return target

    sf = np.asarray(slot_features, np.float32)
    sf_n = sf / (np.linalg.norm(sf, axis=-1, keepdims=True) + np.float32(1e-8))
    ci_a = np.array(ci_l)
    cj_a = np.array(cj_l)
    rel_i = ci_a % mpw - 1
    rel_j = cj_a % mpw - 1
    fi = sf_n[np.array(wi_l), rel_i]
    fj = sf_n[np.array(wj_l), rel_j]
    sims = np.sum(fi * fj, axis=-1)
    hz = np.array(hz_l)
    edge_ok = np.where(hz, edge_r[ci_a] & edge_l[cj_a], edge_b[ci_a] & edge_t[cj_a])
    passing = edge_ok & (sims > np.float32(SIM_THRESH))

    merged = np.zeros(Ch, bool)
    for ci, cj, ok in zip(ci_l, cj_l, passing):
        if ok and not merged[ci] and not merged[cj]:
            keep, rem = min(ci, cj), max(ci, cj)
            target[target == rem] = keep
            merged[rem] = True
    return target


# --------------------------------------------------------------------------
# wait-split post-pass: the pinned neuronxcc allows only ONE sync wait per
# instruction; hoist extras onto preceding same-engine EventSemaphore insts.
# --------------------------------------------------------------------------
def _split_excess_waits(bir_json_bytes, limit=1):
    j = json.loads(bir_json_bytes)
    counter = [0]
    for fn in j.get("functions", []):
        for bb in fn.get("blocks", []):
            new_insts = []
            for inst in bb.get("instructions", []):
                si = inst.get("sync_info") or {}
                waits = si.get("on_wait") or []
                if len(waits) > limit:
                    extra = waits[: len(waits) - limit]
                    si["on_wait"] = waits[len(waits) - limit:]
                    inst["sync_info"] = si
                    for i in range(0, len(extra), limit):
                        counter[0] += 1
                        new_insts.append({
                            "engine": inst["engine"],
                            "ins": [],
                            "name": f"{inst['name']}_hoistw{counter[0]}",
                            "opcode": "EventSemaphore",
                            "outs": [],
                            "sync_info": {"on_update": [],
                                          "on_wait": extra[i: i + limit]},
                        })
                new_insts.append(inst)
            bb["instructions"] = new_insts
    return json.dumps(j).encode()


def _build_program(remap_key):
    if remap_key in _cache:
        return _cache[remap_key]

    import concourse.bass as bass
    import concourse.tile as tile
    from concourse import mybir

    remap = list(remap_key)
    # out[c] = sum_{d: remap[d]==c} oh0[d]; channels with remap[d] != d are
    # zeroed.  remap is chain-free (fixed point on keeps).
    adds = [(int(remap[d]), d) for d in range(C) if remap[d] != d]
    rems = [d for d in range(C) if remap[d] != d]

    # batch adds: any subset whose keeps AND rems each form an arithmetic
    # progression (nonzero keep stride) can be one multi-channel TT.
    # Greedy: longest APs first, then pair up leftovers.
    def _batch_adds(pairs):
        remaining = sorted(pairs, key=lambda p: p[1])
        groups = []
        # try AP runs of length >= 2 (greedy longest-first)
        while remaining:
            best = None
            n = len(remaining)
            for i in range(n):
                for j in range(i + 1, n):
                    k0, r0 = remaining[i]
                    k1, r1 = remaining[j]
                    sk, sr = k1 - k0, r1 - r0
                    if sk == 0:
                        continue
                    run = [remaining[i], remaining[j]]
                    ck, cr = k1, r1
                    for l in range(j + 1, n):
                        k2, r2 = remaining[l]
                        if k2 == ck + sk and r2 == cr + sr:
                            run.append(remaining[l])
                            ck, cr = k2, r2
                    if best is None or len(run) > len(best[0]):
                        best = (run, sk, sr)
            if best is None or len(best[0]) < 2:
                break
            run, sk, sr = best
            groups.append((run, sk, sr))
            for p in run:
                remaining.remove(p)
        for p in remaining:
            groups.append(([p], 1, 1))
        return groups

    add_groups = _batch_adds(adds)

    # batch zeros: maximal uniform-stride runs over sorted rems
    def _batch_runs(chans):
        chans = sorted(chans)
        groups = []
        i = 0
        while i < len(chans):
            j = i + 1
            stride = None
            while j < len(chans):
                s = chans[j] - chans[j - 1]
                if stride is None:
                    stride = s
                if s != stride:
                    break
                j += 1
            groups.append((chans[i:j], stride if j - i > 1 else 1))
            i = j
        return groups

    zero_groups = _batch_runs(rems)

    f32 = mybir.dt.float32
    nc = bass.Bass()
    masks_in = nc.dram_tensor("masks", [C, ROWS_PER_CORE, IMG_W], f32,
                              kind="ExternalInput")
    out_dram = nc.dram_tensor("out", [C, ROWS_PER_CORE, IMG_W], f32,
                              kind="ExternalOutput")

    def _chan_slice_ap(tile_ap, chans, stride):
        # AP over out_tile channels {chans[0], chans[0]+stride, ...} x [G]
        base = tile_ap[:, chans[0], :]
        ch_stride = base.ap[-1][0] * G * stride
        return bass.AP(tensor=base.tensor, offset=base.offset,
                       ap=[base.ap[0], [ch_stride, len(chans)], base.ap[-1]])

    with tile.TileContext(nc) as tc:
        with (
            tc.tile_pool(name="inp", bufs=3) as inp,
            tc.tile_pool(name="outp", bufs=2) as outp,
            tc.tile_pool(name="work", bufs=1) as work,
        ):
            for t in range(NTILES):
                sl = slice(G * t, G * (t + 1))
                in_tile = inp.tile([128, C, G], f32, tag="in_tile")
                t8a = work.tile([128, 8, G], f32, tag="t8a")
                t8b = work.tile([128, 8, G], f32, tag="t8b")
                m8 = work.tile([128, 8, G], f32, tag="m8")
                m4 = work.tile([128, 4, G], f32, tag="m4")
                m2 = work.tile([128, 2, G], f32, tag="m2")
                mx = work.tile([128, G], f32, tag="mx")
                TT = nc.vector.tensor_tensor
                MAX = mybir.AluOpType.max
                if t == 0:
                    # first tile: quarter loads + quarters-first tree; each
                    # quarter's compute waits only its own DMA, so the DVE
                    # starts as soon as the first 1 MiB lands
                    for q in range(4):
                        cq = slice(8 * q, 8 * (q + 1))
                        nc.sync.dma_start(
                            in_tile[:, cq, :],
                            masks_in[cq, :, sl].rearrange("d p g -> p d g"))
                    q2v = [t8a[:, 0:4, :], t8a[:, 4:8, :],
                           t8b[:, 0:4, :], t8b[:, 4:8, :]]
                    q1v = [m8[:, 0:2, :], m8[:, 2:4, :],
                           m8[:, 4:6, :], m8[:, 6:8, :]]
                    for q in range(4):
                        TT(out=q2v[q], in0=in_tile[:, 8 * q:8 * q + 4, :],
                           in1=in_tile[:, 8 * q + 4:8 * q + 8, :], op=MAX)
                        TT(out=q1v[q], in0=q2v[q][:, 0:2, :],
                           in1=q2v[q][:, 2:4, :], op=MAX)
                        TT(out=m4[:, q, :], in0=q1v[q][:, 0, :],
                           in1=q1v[q][:, 1, :], op=MAX)
                    TT(out=m2[:], in0=m4[:, 0:2, :], in1=m4[:, 2:4, :], op=MAX)
                    TT(out=mx[:], in0=m2[:, 0, :], in1=m2[:, 1, :], op=MAX)
                else:
                    nc.sync.dma_start(
                        in_tile[:],
                        masks_in[:, :, sl].rearrange("d p g -> p d g"))

                    # pairwise max tree (contiguous innermost)
                    TT(out=t8a[:], in0=in_tile[:, 0:8, :],
                       in1=in_tile[:, 8:16, :], op=MAX)
                    TT(out=t8b[:], in0=in_tile[:, 16:24, :],
                       in1=in_tile[:, 24:32, :], op=MAX)
                    TT(out=m8[:], in0=t8a[:], in1=t8b[:], op=MAX)
                    TT(out=m4[:], in0=m8[:, 0:4, :], in1=m8[:, 4:8, :], op=MAX)
                    TT(out=m2[:], in0=m4[:, 0:2, :], in1=m4[:, 2:4, :], op=MAX)
                    TT(out=mx[:], in0=m2[:, 0, :], in1=m2[:, 1, :], op=MAX)

                # one-hot: is_equal against broadcast max (exact f32 compare;
                # host pre-pass guarantees a unique per-pixel winner)
                out_tile = outp.tile([128, C, G], f32, tag="out_tile")
                mx_ap = mx[:]
                mx_b = bass.AP(tensor=mx_ap.tensor, offset=mx_ap.offset,
                               ap=[mx_ap.ap[0], [0, C], mx_ap.ap[-1]])
                nc.vector.tensor_tensor(out=out_tile[:], in0=in_tile[:],
                                        in1=mx_b,
                                        op=mybir.AluOpType.is_equal)

                # channel merges (baked in from remap), batched by stride
                for pairs, sk, sr in add_groups:
                    keeps = [p[0] for p in pairs]
                    rms = [p[1] for p in pairs]
                    kap = _chan_slice_ap(out_tile, keeps, sk)
                    rap = _chan_slice_ap(out_tile, rms, sr)
                    nc.vector.tensor_tensor(out=kap, in0=kap, in1=rap,
                                            op=mybir.AluOpType.add)
                for chans, stride in zero_groups:
                    zap = _chan_slice_ap(out_tile, chans, stride)
                    nc.scalar.activation(
                        zap, zap,
                        mybir.ActivationFunctionType.Identity, scale=0.0)

                if t == NTILES - 1:
                    # last tile: channel-quartered stores so the tail is
                    # gen+transfer of 1 MiB pieces instead of one 4 MiB blob
                    for q in range(4):
                        cq = slice(8 * q, 8 * (q + 1))
                        nc.sync.dma_start(
                            out_dram[cq, :, sl].rearrange("c p g -> p c g"),
                            out_tile[:, cq, :])
                else:
                    nc.sync.dma_start(
                        out_dram[:, :, sl].rearrange("c p g -> p c g"),
                        out_tile[:])

    orig = nc.to_json_bytes
    nc.to_json_bytes = lambda: _split_excess_waits(orig())
    _cache[remap_key] = nc
    return nc


def kernel(masks, slot_features, pad_left, pad_top):
    from concourse.bass_utils import run_bass_kernel_spmd

    masks = np.asarray(masks, np.float32)
    slot_features = np.asarray(slot_features, np.float32)
    pl = [int(v) for v in np.asarray(pad_left)]
    pt = [int(v) for v in np.asarray(pad_top)]

    remap = _compute_remap(masks, slot_features, pl, pt)

    # tie pre-fix: where >1 channel equals the per-pixel max, bump the first
    # (= reference argmax winner) by 1 ulp so the device is_equal single-fires
    m0 = masks[0]
    mxh = m0.max(axis=0)
    eq = m0 == mxh[None]
    nties = int((eq.sum(axis=0) > 1).sum())
    if nties:
        masks = masks.copy()
        m0 = masks[0]
        ys, xs = np.nonzero(eq.sum(axis=0) > 1)
        for y, x in zip(ys, xs):
            d0 = int(np.argmax(eq[:, y, x]))
            v = m0[d0, y, x]
            m0[d0, y, x] = np.nextafter(v, np.float32(np.inf), dtype=np.float32)

    nc = _build_program(tuple(int(v) for v in remap))
    in_maps = []
    for i in range(N_CORES):
        slab = np.ascontiguousarray(
            masks[0, :, i * ROWS_PER_CORE:(i + 1) * ROWS_PER_CORE, :])
        in_maps.append({"masks": slab})

    res = run_bass_kernel_spmd(nc, in_maps, core_ids=list(range(N_CORES)))

    out = np.empty((1, C, IMG_H, IMG_W), np.float32)
    for i, r in enumerate(res.results):
        out[0, :, i * ROWS_PER_CORE:(i + 1) * ROWS_PER_CORE, :] = r["out"]
    return out
